# revision 14
# baseline (speedup 1.0000x reference)
"""Causal self-attention on 8 TRN2 NeuronCores.

Sharding: 4-way data parallel over batch x 2-way tensor parallel over heads.
Core c handles batch b=c//2, head group g=c%2 (heads 8g..8g+8).

Per-core device kernel (bf16 matmuls, fp32 PSUM):
  1. QKV projection from host-pretransposed xT [C, T]:
     qT/kT head-dim-on-partitions ([128, T] tiles, head pairs stacked
     64+64 on partitions); V natural [T, 64/head] + ones column (V').
  2. Attention per head-PAIR, q processed in 512-chunks, k-block-major:
     S^T[k,q] for both heads via two row-tiled matmuls (K=64 each, PE
     runs them concurrently); one wide ACT exp (scale=1/8) over both
     heads; diagonal-block causal mask multiplied on DVE;
     Y'[65, q-chunk] += V'_j.T @ expS^T accumulates unnormalized y^T and
     the softmax denominator l (ones column).
     Normalize: DVE recip of l -> DMA partition-broadcast -> DVE mul
     writes y^T straight into the persistent proj-lhsT tiles yf.
  3. proj partial[q, :] = yT.T @ w_proj(group rows) + bp_eff (bp_eff
     host-folds b_proj/2 and the V-bias contribution bv @ w_proj).
  4. Pairwise ReduceScatter(add, f32) sums the two head groups and
     writes each core's query half DIRECTLY into the output tensor.

QKV/V/proj matmul groups are emitted as small "filler" units drained
between attention steps so the PE stays busy under the ACT-bound
exp stream.
"""
import collections
import numpy as np
import ml_dtypes

B, T, C = 4, 2048, 1024
H = 16
D = C // H  # 64
HPC = 8            # heads per core
GD = HPC * D       # 512 dims per core's head group

_CACHE = {}


def _build_nc(skip_rs=False, with_bias=False):
    import concourse.bass as bass
    import concourse.mybir as mybir
    import concourse.tile as tile
    from concourse import bacc
    from contextlib import ExitStack

    f32 = mybir.dt.float32
    bf16 = mybir.dt.bfloat16

    nc = bacc.Bacc("TRN2", target_bir_lowering=False, debug=False, num_devices=8)

    xT = nc.declare_dram_parameter("xT", [C, T], bf16, isOutput=False)
    wq = nc.declare_dram_parameter("wq", [C, GD], bf16, isOutput=False)
    wk = nc.declare_dram_parameter("wk", [C, GD], bf16, isOutput=False)
    wv = nc.declare_dram_parameter("wv", [C, GD], bf16, isOutput=False)
    wp = nc.declare_dram_parameter("wp", [GD, C], bf16, isOutput=False)
    bp = nc.declare_dram_parameter("bp", [C], f32, isOutput=False)
    if with_bias:
        bq = nc.declare_dram_parameter("bq", [GD], f32, isOutput=False)
        bk = nc.declare_dram_parameter("bk", [GD], f32, isOutput=False)
    out = nc.declare_dram_parameter("out", [T // 2, C], f32, isOutput=True)

    rs_in = nc.dram_tensor("rs_in", [T, C], bf16)
    rs_out = nc.dram_tensor("rs_out", [T // 2, C], bf16)

    NKB = T // 128   # 16 k-blocks
    NCC = C // 128   # 8 contraction chunks

    with tile.TileContext(nc) as tc, ExitStack() as S0:
        consts = S0.enter_context(tc.tile_pool(name="consts", bufs=1))
        wqkv = S0.enter_context(tc.tile_pool(name="wqkv", bufs=1))
        xp = S0.enter_context(tc.tile_pool(name="xp", bufs=1))
        wpp = S0.enter_context(tc.tile_pool(name="wpp", bufs=1))
        qk_pool = S0.enter_context(tc.tile_pool(name="qk", bufs=1))
        v_pool = S0.enter_context(tc.tile_pool(name="v", bufs=1))
        yf_pool = S0.enter_context(tc.tile_pool(name="yf", bufs=1))
        esp = S0.enter_context(tc.tile_pool(name="esp", bufs=3))
        rcp = S0.enter_context(tc.tile_pool(name="rcp", bufs=2))
        obp = S0.enter_context(tc.tile_pool(name="ob", bufs=2))
        odp = S0.enter_context(tc.tile_pool(name="od", bufs=2))
        # PSUM: sps 2x[128,1024]f32 (4 banks) + yps 2x[65,512] (2) + psb 2x[128,512] (2)
        sps = S0.enter_context(tc.tile_pool(name="sps", bufs=2, space="PSUM"))
        yps = S0.enter_context(tc.tile_pool(name="yps", bufs=1, space="PSUM"))
        psb = S0.enter_context(tc.tile_pool(name="psb", bufs=2, space="PSUM"))

        # ---- constants ----
        mask01 = consts.tile([128, 128], bf16, tag="mask")
        ones_t = consts.tile([128, D], bf16, tag="ones")
        nc.vector.memset(ones_t, 1.0)
        nc.gpsimd.memset(mask01, 1.0)
        # S^T[k, q] valid when k <= q: zero the strict lower triangle (k > q),
        # applied multiplicatively AFTER exp.
        nc.gpsimd.affine_select(
            out=mask01, in_=mask01,
            compare_op=mybir.AluOpType.is_ge, fill=0.0,
            base=0, pattern=[[1, 128]], channel_multiplier=-1,
        )
        bp_bc = consts.tile([128, C], f32, tag="bpb")
        nc.sync.dma_start(out=bp_bc, in_=bp.ap().partition_broadcast(128))
        if with_bias:
            bq_t = consts.tile([128, 4], f32, tag="bqt")
            bk_t = consts.tile([128, 4], f32, tag="bkt")
            for p in range(4):
                nc.sync.dma_start(
                    out=bq_t[:, p : p + 1],
                    in_=bq.ap()[128 * p : 128 * p + 128].rearrange("(p o) -> p o", o=1),
                )
                nc.sync.dma_start(
                    out=bk_t[:, p : p + 1],
                    in_=bk.ap()[128 * p : 128 * p + 128].rearrange("(p o) -> p o", o=1),
                )
            bqb = consts.tile([128, 4, 512], f32, tag="bqb")
            bkb = consts.tile([128, 4, 512], f32, tag="bkb")
            nc.vector.memset(bqb, 0.0)
            nc.vector.memset(bkb, 0.0)
            for p in range(4):
                nc.vector.tensor_scalar_add(bqb[:, p, :], bqb[:, p, :], bq_t[:, p : p + 1])
                nc.vector.tensor_scalar_add(bkb[:, p, :], bkb[:, p, :], bk_t[:, p : p + 1])

        # ---- persistent tiles ----
        wq_t = [wqkv.tile([128, GD], bf16, tag=f"wq{i}", name=f"wqt{i}") for i in range(NCC)]
        wk_t = [wqkv.tile([128, GD], bf16, tag=f"wk{i}", name=f"wkt{i}") for i in range(NCC)]
        wv_t = [wqkv.tile([128, GD], bf16, tag=f"wv{i}", name=f"wvt{i}") for i in range(NCC)]
        xT_t = [xp.tile([128, T], bf16, tag=f"x{i}", name=f"x{i}") for i in range(NCC)]
        wp_t = [wpp.tile([128, C], bf16, tag=f"wp{i}", name=f"wp{i}") for i in range(4)]
        qT = [qk_pool.tile([128, T], bf16, tag=f"qT{p}", name=f"qT{p}") for p in range(4)]
        kT = [qk_pool.tile([128, T], bf16, tag=f"kT{p}", name=f"kT{p}") for p in range(4)]
        vp = [v_pool.tile([128, HPC * 65], bf16, tag=f"vp{tb}", name=f"vp{tb}") for tb in range(NKB)]
        yf = [yf_pool.tile([128, T], bf16, tag=f"yf{p}", name=f"yf{p}") for p in range(4)]

        for i in range(NCC):
            sl = slice(128 * i, 128 * i + 128)
            nc.sync.dma_start(out=wq_t[i], in_=wq.ap()[sl, :])
            nc.sync.dma_start(out=wk_t[i], in_=wk.ap()[sl, :])
            nc.sync.dma_start(out=xT_t[i], in_=xT.ap()[sl, :])
        for i in range(NCC):
            nc.sync.dma_start(out=wv_t[i], in_=wv.ap()[128 * i : 128 * i + 128, :])
        for i in range(4):
            nc.sync.dma_start(out=wp_t[i], in_=wp.ap()[128 * i : 128 * i + 128, :])
        # ones columns of V' (written once; V evac fills only [0:64] per head)
        for tb in range(NKB):
            nc.vector.memset(
                vp[tb].rearrange("p (h e) -> p h e", e=65)[:, :, D : D + 1], 1.0
            )

        # ---- emission thunk generators (filler units) ----
        def qkT_group(is_k, p, t4):
            """One [128,512] psum group of the q/k projection -> 9 thunks."""
            w_t = wk_t if is_k else wq_t
            dst = (kT if is_k else qT)[p]
            cell = {}

            def mk_mm(cc):
                def f():
                    if cc == 0:
                        cell["ps"] = psb.tile([128, 512], f32, tag="pf", name="pf")
                    nc.tensor.matmul(
                        cell["ps"],
                        w_t[cc][:, 128 * p : 128 * p + 128],
                        xT_t[cc][:, 512 * t4 : 512 * t4 + 512],
                        start=(cc == 0), stop=(cc == NCC - 1),
                    )
                return f

            def evac():
                d = dst[:, 512 * t4 : 512 * t4 + 512]
                if with_bias:
                    bb = (bkb if is_k else bqb)[:, p, :]
                    nc.vector.tensor_add(d, cell["ps"], bb)
                else:
                    nc.vector.tensor_copy(d, cell["ps"])

            return [mk_mm(cc) for cc in range(NCC)] + [evac]

        def v_group(tb):
            cell = {}

            def mk_mm(cc):
                def f():
                    if cc == 0:
                        cell["ps"] = psb.tile([128, GD], f32, tag="pf", name="pv")
                    nc.tensor.matmul(
                        cell["ps"],
                        xT_t[cc][:, 128 * tb : 128 * tb + 128],
                        wv_t[cc],
                        start=(cc == 0), stop=(cc == NCC - 1),
                    )
                return f

            def evac():
                v3 = vp[tb].rearrange("p (h e) -> p h e", e=65)
                nc.vector.tensor_copy(
                    v3[:, :, 0:D], cell["ps"].rearrange("p (h e) -> p h e", e=D)
                )

            return [mk_mm(cc) for cc in range(NCC)] + [evac]

        def proj_group(qq):
            """qq is the GLOBAL 128-row q block (0..15); 2 psum halves."""
            thunks = []
            cell = {}

            def alloc_ob():
                cell["ob"] = obp.tile([128, C], bf16, tag="ob", name="ob")

            for cc2 in range(2):
                def mk_mm(dd, cc2=cc2):
                    def f():
                        if dd == 0:
                            if cc2 == 0:
                                alloc_ob()
                            cell["ps"] = psb.tile([128, 512], f32, tag="pf", name="pp")
                        nc.tensor.matmul(
                            cell["ps"],
                            yf[dd][:, 128 * qq : 128 * qq + 128],
                            wp_t[dd][:, 512 * cc2 : 512 * cc2 + 512],
                            start=(dd == 0), stop=(dd == 3),
                        )
                    return f

                def evac(cc2=cc2):
                    nc.vector.tensor_add(
                        cell["ob"][:, 512 * cc2 : 512 * cc2 + 512],
                        cell["ps"],
                        bp_bc[:, 512 * cc2 : 512 * cc2 + 512],
                    )

                thunks += [mk_mm(dd) for dd in range(4)] + [evac]

            def dma():
                nc.sync.dma_start(
                    out=rs_in.ap()[128 * qq : 128 * qq + 128, :], in_=cell["ob"]
                )

            thunks.append(dma)
            return thunks

        # drain pulls from the first non-empty deque in drain_sources
        fillers = collections.deque()
        drain_sources = [fillers]

        def drain(n):
            for _ in range(n):
                for q in drain_sources:
                    if q:
                        q.popleft()()
                        break
                else:
                    return

        def drain_all():
            for q in drain_sources:
                while q:
                    q.popleft()()

        # ---- attention ----
        def attn_pair(m, p, after_cl=None):
            h0, h1 = 2 * p, 2 * p + 1
            for cl in (2 * m, 2 * m + 1):
                Y0 = yps.tile([65, 512], f32, tag="yh", name="yh")
                Y1 = yps.tile([65, 512], f32, tag="yh2", name="yh2")
                nj = 4 * cl + 4
                prev = None  # (es, j, qa_l, w)
                for j in range(nj):
                    qa_l = max(0, 128 * j - 512 * cl)
                    w = 512 - qa_l
                    qsl = slice(512 * cl + qa_l, 512 * cl + 512)
                    ksl = slice(128 * j, 128 * j + 128)
                    st = sps.tile([128, 1024], f32, tag="s", name="st")
                    nc.tensor.matmul(
                        st[:, 0:w], kT[p][0:64, ksl], qT[p][0:64, qsl],
                        start=True, stop=True,
                    )
                    nc.tensor.matmul(
                        st[:, 512 : 512 + w], kT[p][64:128, ksl], qT[p][64:128, qsl],
                        start=True, stop=True,
                    )
                    es = esp.tile([128, 1024], bf16, tag="es", name="es")
                    nc.scalar.activation(
                        es.rearrange("pp (h q) -> pp h q", h=2)[:, :, 0:w],
                        st.rearrange("pp (h q) -> pp h q", h=2)[:, :, 0:w],
                        mybir.ActivationFunctionType.Exp,
                        bias=0.0, scale=0.125,
                    )
                    if j >= 4 * cl:  # diagonal block: first 128 cols of region
                        nc.vector.tensor_mul(es[:, 0:128], es[:, 0:128], mask01)
                        nc.vector.tensor_mul(es[:, 512:640], es[:, 512:640], mask01)
                    drain(1)
                    if prev is not None:
                        emit_av(prev, nj, Y0, Y1, h0, h1)
                        drain(1)
                    prev = (es, j, qa_l, w)
                emit_av(prev, nj, Y0, Y1, h0, h1)
                # normalize both heads for this q-chunk: recip of l, rank-1
                # PE broadcast across the 64 d-partitions, then scale.
                for Y, r in ((Y0, 0), (Y1, 1)):
                    rb = rcp.tile([1, 512], bf16, tag="rb", name="rb")
                    with nc.allow_low_precision(reason="softmax denom bf16"):
                        nc.vector.reciprocal(rb, Y[64:65, :])
                    rbc = sps.tile([64, 512], f32, tag="s", name="rbc")
                    nc.tensor.matmul(
                        rbc, ones_t[0:1, 0:64], rb[0:1, :], start=True, stop=True
                    )
                    rbs = rcp.tile([64, 512], f32, tag="rbs", name="rbs")
                    nc.vector.tensor_copy(rbs, rbc)
                    nc.vector.tensor_mul(
                        yf[p][64 * r : 64 * r + 64, 512 * cl : 512 * cl + 512],
                        Y[0:64, :],
                        rbs,
                    )
                drain(2)
                if after_cl is not None:
                    after_cl(cl)

        def emit_av(prev, nj, Y0, Y1, h0, h1):
            es, j, qa_l, w = prev
            last = j == nj - 1
            nc.tensor.matmul(
                Y0[:, qa_l:512], vp[j][:, 65 * h0 : 65 * h0 + 65], es[:, 0:w],
                start=(j == 0), stop=last, skip_group_check=True,
            )
            nc.tensor.matmul(
                Y1[:, qa_l:512], vp[j][:, 65 * h1 : 65 * h1 + 65], es[:, 512 : 512 + w],
                start=(j == 0), stop=last, skip_group_check=True,
            )

        def emit_rs(m):
            if not skip_rs:
                nc.gpsimd.collective_compute(
                    "ReduceScatter",
                    mybir.AluOpType.add,
                    ins=[rs_in.ap()[1024 * m : 1024 * m + 1024, :]],
                    outs=[rs_out.ap()[512 * m : 512 * m + 512, :]],
                    replica_groups=[[0, 1], [2, 3], [4, 5], [6, 7]],
                )
            # bf16 -> f32 convert-out stage, all on GPSIMD + SWDGE so the
            # compute engines and HWDGE queues stay clear.
            src = rs_out if not skip_rs else rs_in
            for i in range(4):
                r0 = 512 * m + 128 * i
                s0 = r0 if not skip_rs else 1024 * m + 128 * i
                t_bf = odp.tile([128, C], bf16, tag="tbf", name="tbf")
                t_f32 = odp.tile([128, C], f32, tag="tf32", name="tf32")
                nc.gpsimd.dma_start(out=t_bf, in_=src.ap()[s0 : s0 + 128, :])
                nc.gpsimd.tensor_copy(t_f32, t_bf)
                nc.gpsimd.dma_start(out=out.ap()[r0 : r0 + 128, :], in_=t_f32)

        # ---- emission schedule ----
        # Lead-in: q/k for pair 0 and V for the first 8 k-blocks, directly.
        for t in qkT_group(False, 0, 0) + qkT_group(True, 0, 0):
            t()
        for t4 in range(1, 4):
            for t in qkT_group(False, 0, t4) + qkT_group(True, 0, t4):
                t()
        for tb in range(8):
            for t in v_group(tb):
                t()

        # Fillers for the m=0 attention phase: remaining projections.
        # fq[p] (next pair's q/k) is drained with priority over `fillers`.
        fq = {p: collections.deque() for p in (1, 2, 3)}
        for p in (1, 2, 3):
            for t4 in range(4):
                fq[p].extend(qkT_group(False, p, t4))
                fq[p].extend(qkT_group(True, p, t4))
        for tb in range(8, NKB):
            fillers.extend(v_group(tb))

        for p in range(4):
            if p > 0:
                # force-emit anything pair p still needs
                while fq[p]:
                    fq[p].popleft()()
            drain_sources[:] = (
                [fq[p + 1], fillers] if p + 1 in fq else [fillers]
            )
            attn_pair(0, p)

        drain_sources[:] = [fillers]
        drain_all()
        for qq in range(8):
            fillers.extend(proj_group(qq))

        attn_pair(1, 0)
        drain_all()
        emit_rs(0)
        attn_pair(1, 1)
        attn_pair(1, 2)

        def after_cl(cl):
            if cl == 2:
                for qq in range(8, 12):
                    fillers.extend(proj_group(qq))

        attn_pair(1, 3, after_cl=after_cl)
        drain_all()
        for qq in range(12, 16):
            for t in proj_group(qq):
                t()
        emit_rs(1)

    nc.finalize()
    return nc


def get_nc(skip_rs=False, with_bias=False):
    key = ("nc", skip_rs, with_bias)
    if key not in _CACHE:
        _CACHE[key] = _build_nc(skip_rs, with_bias)
    return _CACHE[key]


def build_in_maps(x, w_attn, b_attn, w_proj, b_proj, with_bias=False):
    bf = ml_dtypes.bfloat16
    x = np.asarray(x, dtype=np.float32)
    w_attn = np.asarray(w_attn, dtype=np.float32)
    b_attn = np.asarray(b_attn, dtype=np.float32)
    w_proj = np.asarray(w_proj, dtype=np.float32)
    b_proj = np.asarray(b_proj, dtype=np.float32)

    in_maps = []
    for c in range(8):
        b, g = c // 2, c % 2
        sl = slice(GD * g, GD * g + GD)
        wp_g = w_proj[GD * g : GD * g + GD, :]
        bv_g = b_attn[2 * C :][sl]
        # fold b_proj/2 and the V-bias contribution into one proj bias
        bp_eff = (0.5 * b_proj + bv_g @ wp_g).astype(np.float32)
        m = {
            "xT": np.ascontiguousarray(x[b].T).astype(bf),
            "wq": np.ascontiguousarray(w_attn[:, 0 * C :][:, sl]).astype(bf),
            "wk": np.ascontiguousarray(w_attn[:, 1 * C :][:, sl]).astype(bf),
            "wv": np.ascontiguousarray(w_attn[:, 2 * C :][:, sl]).astype(bf),
            "wp": np.ascontiguousarray(wp_g).astype(bf),
            "bp": bp_eff,
        }
        if with_bias:
            m["bq"] = np.ascontiguousarray(b_attn[0 * C :][sl])
            m["bk"] = np.ascontiguousarray(b_attn[1 * C :][sl])
        in_maps.append(m)
    return in_maps


def assemble_out(results):
    # core with parity g owns q in [512g, 512g+512) of each 1024-half
    out = np.empty((B, T, C), dtype=np.float32)
    for c in range(8):
        b, g = c // 2, c % 2
        piece = results[c]["out"]  # [1024, C]
        out[b, 512 * g : 512 * g + 512, :] = piece[0:512]
        out[b, 1024 + 512 * g : 1024 + 512 * g + 512, :] = piece[512:1024]
    return out


def kernel(x, w_attn, b_attn, w_proj, b_proj):
    from concourse.bass_utils import run_bass_kernel_spmd

    b_attn_np = np.asarray(b_attn, dtype=np.float32)
    with_bias = bool(np.any(b_attn_np[: 2 * C] != 0.0))
    nc = get_nc(with_bias=with_bias)
    in_maps = build_in_maps(x, w_attn, b_attn, w_proj, b_proj, with_bias=with_bias)
    res = run_bass_kernel_spmd(nc, in_maps, core_ids=list(range(8)))
    return assemble_out(res.results)


# revision 19
# speedup vs baseline: 1.9530x; 1.9530x over previous
"""Causal self-attention on 8 TRN2 NeuronCores.

Sharding: 4-way data parallel over batch x 2-way tensor parallel over heads.
Core c handles batch b=c//2, head group g=c%2 (heads 8g..8g+8).

Per-core device kernel (bf16 matmuls, fp32 PSUM):
  1. QKV projection from host-pretransposed xT [C, T]:
     qT/kT head-dim-on-partitions ([128, T] tiles, head pairs stacked
     64+64 on partitions); V natural [T, 64/head] + ones column (V').
  2. Attention per head-PAIR, q processed in 512-chunks, k-block-major:
     S^T[k,q] for both heads via two row-tiled matmuls (K=64 each, PE
     runs them concurrently); one wide ACT exp (scale=1/8) over both
     heads; diagonal-block causal mask multiplied on DVE;
     Y'[65, q-chunk] += V'_j.T @ expS^T accumulates unnormalized y^T and
     the softmax denominator l (ones column).
     Normalize: DVE recip of l -> DMA partition-broadcast -> DVE mul
     writes y^T straight into the persistent proj-lhsT tiles yf.
  3. proj partial[q, :] = yT.T @ w_proj(group rows) + bp_eff (bp_eff
     host-folds b_proj/2 and the V-bias contribution bv @ w_proj).
  4. Pairwise ReduceScatter(add, f32) sums the two head groups and
     writes each core's query half DIRECTLY into the output tensor.

QKV/V/proj matmul groups are emitted as small "filler" units drained
between attention steps so the PE stays busy under the ACT-bound
exp stream.
"""
import collections
import numpy as np
import ml_dtypes

B, T, C = 4, 2048, 1024
H = 16
D = C // H  # 64
HPC = 8            # heads per core
GD = HPC * D       # 512 dims per core's head group

_CACHE = {}


def _build_nc(skip_rs=False, with_bias=False):
    import concourse.bass as bass
    import concourse.mybir as mybir
    import concourse.tile as tile
    from concourse import bacc
    from contextlib import ExitStack

    f32 = mybir.dt.float32
    bf16 = mybir.dt.bfloat16

    nc = bacc.Bacc("TRN2", target_bir_lowering=False, debug=False, num_devices=8)

    xT = nc.declare_dram_parameter("xT", [C, T], bf16, isOutput=False)
    wq = nc.declare_dram_parameter("wq", [C, GD], bf16, isOutput=False)
    wk = nc.declare_dram_parameter("wk", [C, GD], bf16, isOutput=False)
    wv = nc.declare_dram_parameter("wv", [C, GD], bf16, isOutput=False)
    wp = nc.declare_dram_parameter("wp", [GD, C], bf16, isOutput=False)
    bp = nc.declare_dram_parameter("bp", [C], f32, isOutput=False)
    if with_bias:
        bq = nc.declare_dram_parameter("bq", [GD], f32, isOutput=False)
        bk = nc.declare_dram_parameter("bk", [GD], f32, isOutput=False)
    out = nc.declare_dram_parameter("out", [T // 2, C], f32, isOutput=True)

    rs_in = nc.dram_tensor("rs_in", [T, C], bf16)
    rs_out = nc.dram_tensor("rs_out", [T // 2, C], bf16)

    NKB = T // 128   # 16 k-blocks
    NCC = C // 128   # 8 contraction chunks

    with tile.TileContext(nc) as tc, ExitStack() as S0:
        consts = S0.enter_context(tc.tile_pool(name="consts", bufs=1))
        wqkv = S0.enter_context(tc.tile_pool(name="wqkv", bufs=1))
        xp = S0.enter_context(tc.tile_pool(name="xp", bufs=1))
        wpp = S0.enter_context(tc.tile_pool(name="wpp", bufs=1))
        qk_pool = S0.enter_context(tc.tile_pool(name="qk", bufs=1))
        v_pool = S0.enter_context(tc.tile_pool(name="v", bufs=1))
        yf_pool = S0.enter_context(tc.tile_pool(name="yf", bufs=1))
        esp = S0.enter_context(tc.tile_pool(name="esp", bufs=3))
        rcp = S0.enter_context(tc.tile_pool(name="rcp", bufs=2))
        obp = S0.enter_context(tc.tile_pool(name="ob", bufs=2))
        odp = S0.enter_context(tc.tile_pool(name="od", bufs=2))
        # PSUM: sps 2x[128,1024]f32 (4 banks) + yps 2x[65,512] (2) + psb 2x[128,512] (2)
        sps = S0.enter_context(tc.tile_pool(name="sps", bufs=2, space="PSUM"))
        yps = S0.enter_context(tc.tile_pool(name="yps", bufs=1, space="PSUM"))
        psb = S0.enter_context(tc.tile_pool(name="psb", bufs=2, space="PSUM"))

        # ---- constants ----
        mask01 = consts.tile([128, 128], bf16, tag="mask")
        ones_t = consts.tile([128, D], bf16, tag="ones")
        nc.vector.memset(ones_t, 1.0)
        nc.gpsimd.memset(mask01, 1.0)
        # S^T[k, q] valid when k <= q: zero the strict lower triangle (k > q),
        # applied multiplicatively AFTER exp.
        nc.gpsimd.affine_select(
            out=mask01, in_=mask01,
            compare_op=mybir.AluOpType.is_ge, fill=0.0,
            base=0, pattern=[[1, 128]], channel_multiplier=-1,
        )
        bp_bc = consts.tile([128, C], f32, tag="bpb")
        nc.sync.dma_start(out=bp_bc, in_=bp.ap().partition_broadcast(128))
        if with_bias:
            bq_t = consts.tile([128, 4], f32, tag="bqt")
            bk_t = consts.tile([128, 4], f32, tag="bkt")
            for p in range(4):
                nc.sync.dma_start(
                    out=bq_t[:, p : p + 1],
                    in_=bq.ap()[128 * p : 128 * p + 128].rearrange("(p o) -> p o", o=1),
                )
                nc.sync.dma_start(
                    out=bk_t[:, p : p + 1],
                    in_=bk.ap()[128 * p : 128 * p + 128].rearrange("(p o) -> p o", o=1),
                )
            bqb = consts.tile([128, 4, 512], f32, tag="bqb")
            bkb = consts.tile([128, 4, 512], f32, tag="bkb")
            nc.vector.memset(bqb, 0.0)
            nc.vector.memset(bkb, 0.0)
            for p in range(4):
                nc.vector.tensor_scalar_add(bqb[:, p, :], bqb[:, p, :], bq_t[:, p : p + 1])
                nc.vector.tensor_scalar_add(bkb[:, p, :], bkb[:, p, :], bk_t[:, p : p + 1])

        # ---- persistent tiles ----
        wq_t = [wqkv.tile([128, GD], bf16, tag=f"wq{i}", name=f"wqt{i}") for i in range(NCC)]
        wk_t = [wqkv.tile([128, GD], bf16, tag=f"wk{i}", name=f"wkt{i}") for i in range(NCC)]
        wv_t = [wqkv.tile([128, GD], bf16, tag=f"wv{i}", name=f"wvt{i}") for i in range(NCC)]
        xT_t = [xp.tile([128, T], bf16, tag=f"x{i}", name=f"x{i}") for i in range(NCC)]
        wp_t = [wpp.tile([128, C], bf16, tag=f"wp{i}", name=f"wp{i}") for i in range(4)]
        qT = [qk_pool.tile([128, T], bf16, tag=f"qT{p}", name=f"qT{p}") for p in range(4)]
        kT = [qk_pool.tile([128, T], bf16, tag=f"kT{p}", name=f"kT{p}") for p in range(4)]
        vp = [v_pool.tile([128, HPC * 65], bf16, tag=f"vp{tb}", name=f"vp{tb}") for tb in range(NKB)]
        yf = [yf_pool.tile([128, T], bf16, tag=f"yf{p}", name=f"yf{p}") for p in range(4)]

        # x columns [0:1024] unblock qkT(0) t4=0,1 and V(0..7); the rest
        # streams in behind them.
        for i in range(NCC):
            sl = slice(128 * i, 128 * i + 128)
            nc.sync.dma_start(out=wq_t[i], in_=wq.ap()[sl, :])
            nc.sync.dma_start(out=wk_t[i], in_=wk.ap()[sl, :])
            nc.sync.dma_start(out=xT_t[i][:, 0:1024], in_=xT.ap()[sl, 0:1024])
        for i in range(NCC):
            sl = slice(128 * i, 128 * i + 128)
            nc.sync.dma_start(out=wv_t[i], in_=wv.ap()[sl, :])
            nc.sync.dma_start(out=xT_t[i][:, 1024:2048], in_=xT.ap()[sl, 1024:2048])
        for i in range(4):
            nc.sync.dma_start(out=wp_t[i], in_=wp.ap()[128 * i : 128 * i + 128, :])
        # ones columns of V' (written once; V evac fills only [0:64] per head)
        for tb in range(NKB):
            nc.vector.memset(
                vp[tb].rearrange("p (h e) -> p h e", e=65)[:, :, D : D + 1], 1.0
            )

        # ---- emission thunk generators (filler units) ----
        def qkT_group(is_k, p, t4):
            """One [128,512] psum group of the q/k projection -> 9 thunks."""
            w_t = wk_t if is_k else wq_t
            dst = (kT if is_k else qT)[p]
            cell = {}

            def mk_mm(cc):
                def f():
                    if cc == 0:
                        cell["ps"] = psb.tile([128, 512], f32, tag="pf", name="pf")
                    nc.tensor.matmul(
                        cell["ps"],
                        w_t[cc][:, 128 * p : 128 * p + 128],
                        xT_t[cc][:, 512 * t4 : 512 * t4 + 512],
                        start=(cc == 0), stop=(cc == NCC - 1),
                    )
                return f

            def evac():
                d = dst[:, 512 * t4 : 512 * t4 + 512]
                if with_bias:
                    bb = (bkb if is_k else bqb)[:, p, :]
                    nc.vector.tensor_add(d, cell["ps"], bb)
                else:
                    nc.vector.tensor_copy(d, cell["ps"])

            return [mk_mm(cc) for cc in range(NCC)] + [evac]

        def v_group(tb):
            cell = {}

            def mk_mm(cc):
                def f():
                    if cc == 0:
                        cell["ps"] = psb.tile([128, GD], f32, tag="pf", name="pv")
                    nc.tensor.matmul(
                        cell["ps"],
                        xT_t[cc][:, 128 * tb : 128 * tb + 128],
                        wv_t[cc],
                        start=(cc == 0), stop=(cc == NCC - 1),
                    )
                return f

            def evac():
                v3 = vp[tb].rearrange("p (h e) -> p h e", e=65)
                nc.vector.tensor_copy(
                    v3[:, :, 0:D], cell["ps"].rearrange("p (h e) -> p h e", e=D)
                )

            return [mk_mm(cc) for cc in range(NCC)] + [evac]

        def proj_group(qq):
            """qq is the GLOBAL 128-row q block (0..15); 2 psum halves."""
            thunks = []
            cell = {}

            def alloc_ob():
                cell["ob"] = obp.tile([128, C], bf16, tag="ob", name="ob")

            for cc2 in range(2):
                def mk_mm(dd, cc2=cc2):
                    def f():
                        if dd == 0:
                            if cc2 == 0:
                                alloc_ob()
                            cell["ps"] = psb.tile([128, 512], f32, tag="pf", name="pp")
                        nc.tensor.matmul(
                            cell["ps"],
                            yf[dd][:, 128 * qq : 128 * qq + 128],
                            wp_t[dd][:, 512 * cc2 : 512 * cc2 + 512],
                            start=(dd == 0), stop=(dd == 3),
                        )
                    return f

                def evac(cc2=cc2):
                    nc.vector.tensor_add(
                        cell["ob"][:, 512 * cc2 : 512 * cc2 + 512],
                        cell["ps"],
                        bp_bc[:, 512 * cc2 : 512 * cc2 + 512],
                    )

                thunks += [mk_mm(dd) for dd in range(4)] + [evac]

            def dma():
                nc.sync.dma_start(
                    out=rs_in.ap()[128 * qq : 128 * qq + 128, :], in_=cell["ob"]
                )

            thunks.append(dma)
            return thunks

        # drain pulls from the first non-empty deque in drain_sources
        fillers = collections.deque()
        drain_sources = [fillers]

        def drain(n):
            for _ in range(n):
                for q in drain_sources:
                    if q:
                        q.popleft()()
                        break
                else:
                    return

        def drain_all():
            for q in drain_sources:
                while q:
                    q.popleft()()

        # ---- attention ----
        def attn_pair(m, p, after_cl=None):
            h0, h1 = 2 * p, 2 * p + 1
            for cl in (2 * m, 2 * m + 1):
                Y0 = yps.tile([65, 512], f32, tag="yh", name="yh")
                Y1 = yps.tile([65, 512], f32, tag="yh2", name="yh2")
                nj = 4 * cl + 4
                prev = None  # (es, j, qa_l, w)
                for j in range(nj):
                    qa_l = max(0, 128 * j - 512 * cl)
                    w = 512 - qa_l
                    qsl = slice(512 * cl + qa_l, 512 * cl + 512)
                    ksl = slice(128 * j, 128 * j + 128)
                    st = sps.tile([128, 1024], f32, tag="s", name="st")
                    nc.tensor.matmul(
                        st[:, 0:w], kT[p][0:64, ksl], qT[p][0:64, qsl],
                        start=True, stop=True,
                    )
                    nc.tensor.matmul(
                        st[:, 512 : 512 + w], kT[p][64:128, ksl], qT[p][64:128, qsl],
                        start=True, stop=True,
                    )
                    es = esp.tile([128, 1024], bf16, tag="es", name="es")
                    nc.scalar.activation(
                        es.rearrange("pp (h q) -> pp h q", h=2)[:, :, 0:w],
                        st.rearrange("pp (h q) -> pp h q", h=2)[:, :, 0:w],
                        mybir.ActivationFunctionType.Exp,
                        bias=0.0, scale=0.125,
                    )
                    if j >= 4 * cl:  # diagonal block: first 128 cols of region
                        nc.vector.tensor_mul(es[:, 0:128], es[:, 0:128], mask01)
                        nc.vector.tensor_mul(es[:, 512:640], es[:, 512:640], mask01)
                    drain(1)
                    if prev is not None:
                        emit_av(prev, nj, Y0, Y1, h0, h1)
                        drain(1)
                    prev = (es, j, qa_l, w)
                emit_av(prev, nj, Y0, Y1, h0, h1)
                # normalize both heads for this q-chunk: recip of l, rank-1
                # PE broadcast across the 64 d-partitions, then scale.
                for Y, r in ((Y0, 0), (Y1, 1)):
                    rb = rcp.tile([1, 512], bf16, tag="rb", name="rb")
                    with nc.allow_low_precision(reason="softmax denom bf16"):
                        nc.vector.reciprocal(rb, Y[64:65, :])
                    rbc = sps.tile([64, 512], f32, tag="s", name="rbc")
                    nc.tensor.matmul(
                        rbc, ones_t[0:1, 0:64], rb[0:1, :], start=True, stop=True
                    )
                    rbs = rcp.tile([64, 512], f32, tag="rbs", name="rbs")
                    nc.vector.tensor_copy(rbs, rbc)
                    nc.vector.tensor_mul(
                        yf[p][64 * r : 64 * r + 64, 512 * cl : 512 * cl + 512],
                        Y[0:64, :],
                        rbs,
                    )
                drain(2)
                if after_cl is not None:
                    after_cl(cl)

        def emit_av(prev, nj, Y0, Y1, h0, h1):
            es, j, qa_l, w = prev
            last = j == nj - 1
            nc.tensor.matmul(
                Y0[:, qa_l:512], vp[j][:, 65 * h0 : 65 * h0 + 65], es[:, 0:w],
                start=(j == 0), stop=last, skip_group_check=True,
            )
            nc.tensor.matmul(
                Y1[:, qa_l:512], vp[j][:, 65 * h1 : 65 * h1 + 65], es[:, 512 : 512 + w],
                start=(j == 0), stop=last, skip_group_check=True,
            )

        def emit_rs(m, h):
            """Quarter ReduceScatter: input rows [1024m+512h, +512), each
            rank keeps 256 rows, stored at out rows [256*(2m+h), +256)."""
            q = 2 * m + h
            if not skip_rs:
                nc.gpsimd.collective_compute(
                    "ReduceScatter",
                    mybir.AluOpType.add,
                    ins=[rs_in.ap()[1024 * m + 512 * h : 1024 * m + 512 * h + 512, :]],
                    outs=[rs_out.ap()[256 * q : 256 * q + 256, :]],
                    replica_groups=[[0, 1], [2, 3], [4, 5], [6, 7]],
                )
            # bf16 -> f32 convert-out stage, all on GPSIMD + SWDGE so the
            # compute engines and HWDGE queues stay clear.
            for i in range(2):
                r0 = 256 * q + 128 * i
                s0 = r0 if not skip_rs else 1024 * m + 512 * h + 128 * i
                src = rs_out if not skip_rs else rs_in
                t_bf = odp.tile([128, C], bf16, tag="tbf", name="tbf")
                t_f32 = odp.tile([128, C], f32, tag="tf32", name="tf32")
                nc.gpsimd.dma_start(out=t_bf, in_=src.ap()[s0 : s0 + 128, :])
                nc.gpsimd.tensor_copy(t_f32, t_bf)
                nc.gpsimd.dma_start(out=out.ap()[r0 : r0 + 128, :], in_=t_f32)

        # ---- emission schedule ----
        # Lead-in: q/k cols [0:1024] for pair 0 and V for the first 8
        # k-blocks — exactly what attn(0,0) consumes.
        for t4 in (0, 1):
            for t in qkT_group(False, 0, t4) + qkT_group(True, 0, t4):
                t()
        for tb in range(8):
            for t in v_group(tb):
                t()

        # Fillers for the m=0 attention phase. attn(0,p) needs only pair
        # p's t4=0,1 (fq[p], drained with priority); the t4=2,3 halves and
        # V(8..15) are only needed for m=1 and fill PE gaps.
        fq = {p: collections.deque() for p in (1, 2, 3)}
        for p in (1, 2, 3):
            for t4 in (0, 1):
                fq[p].extend(qkT_group(False, p, t4))
                fq[p].extend(qkT_group(True, p, t4))
        for tb in range(8, NKB):
            fillers.extend(v_group(tb))
        for p in range(4):
            for t4 in (2, 3):
                fillers.extend(qkT_group(False, p, t4))
                fillers.extend(qkT_group(True, p, t4))

        for p in range(4):
            if p > 0:
                # force-emit anything pair p still needs
                while fq[p]:
                    fq[p].popleft()()
            drain_sources[:] = (
                [fq[p + 1], fillers] if p + 1 in fq else [fillers]
            )
            attn_pair(0, p)

        drain_sources[:] = [fillers]
        drain_all()
        for qq in range(7):
            fillers.extend(proj_group(qq))

        attn_pair(1, 0)
        drain_all()
        emit_rs(0, 0)
        fillers.extend(proj_group(7))
        attn_pair(1, 1)
        drain_all()
        emit_rs(0, 1)
        attn_pair(1, 2)

        def after_cl(cl):
            if cl == 2:
                for qq in range(8, 12):
                    fillers.extend(proj_group(qq))
                # fires from the filler stream mid-cl3, right after the
                # qq8..11 partials land in rs_in
                fillers.append(lambda: emit_rs(1, 0))

        attn_pair(1, 3, after_cl=after_cl)
        drain_all()
        for qq in range(12, 16):
            for t in proj_group(qq):
                t()
        emit_rs(1, 1)

    nc.finalize()
    return nc


def get_nc(skip_rs=False, with_bias=False):
    key = ("nc", skip_rs, with_bias)
    if key not in _CACHE:
        _CACHE[key] = _build_nc(skip_rs, with_bias)
    return _CACHE[key]


def build_in_maps(x, w_attn, b_attn, w_proj, b_proj, with_bias=False):
    bf = ml_dtypes.bfloat16
    x = np.asarray(x, dtype=np.float32)
    w_attn = np.asarray(w_attn, dtype=np.float32)
    b_attn = np.asarray(b_attn, dtype=np.float32)
    w_proj = np.asarray(w_proj, dtype=np.float32)
    b_proj = np.asarray(b_proj, dtype=np.float32)

    in_maps = []
    for c in range(8):
        b, g = c // 2, c % 2
        sl = slice(GD * g, GD * g + GD)
        wp_g = w_proj[GD * g : GD * g + GD, :]
        bv_g = b_attn[2 * C :][sl]
        # fold b_proj/2 and the V-bias contribution into one proj bias
        bp_eff = (0.5 * b_proj + bv_g @ wp_g).astype(np.float32)
        m = {
            "xT": np.ascontiguousarray(x[b].T).astype(bf),
            "wq": np.ascontiguousarray(w_attn[:, 0 * C :][:, sl]).astype(bf),
            "wk": np.ascontiguousarray(w_attn[:, 1 * C :][:, sl]).astype(bf),
            "wv": np.ascontiguousarray(w_attn[:, 2 * C :][:, sl]).astype(bf),
            "wp": np.ascontiguousarray(wp_g).astype(bf),
            "bp": bp_eff,
        }
        if with_bias:
            m["bq"] = np.ascontiguousarray(b_attn[0 * C :][sl])
            m["bk"] = np.ascontiguousarray(b_attn[1 * C :][sl])
        in_maps.append(m)
    return in_maps


def assemble_out(results):
    # quarter-RS: piece rows [256*(2m+h), +256) = global q rows
    # [1024m + 512h + 256g, +256) for core parity g
    out = np.empty((B, T, C), dtype=np.float32)
    for c in range(8):
        b, g = c // 2, c % 2
        piece = results[c]["out"]  # [1024, C]
        for m in range(2):
            for h in range(2):
                q0 = 1024 * m + 512 * h + 256 * g
                r0 = 256 * (2 * m + h)
                out[b, q0 : q0 + 256, :] = piece[r0 : r0 + 256]
    return out


def kernel(x, w_attn, b_attn, w_proj, b_proj):
    from concourse.bass_utils import run_bass_kernel_spmd

    b_attn_np = np.asarray(b_attn, dtype=np.float32)
    with_bias = bool(np.any(b_attn_np[: 2 * C] != 0.0))
    nc = get_nc(with_bias=with_bias)
    in_maps = build_in_maps(x, w_attn, b_attn, w_proj, b_proj, with_bias=with_bias)
    res = run_bass_kernel_spmd(nc, in_maps, core_ids=list(range(8)))
    return assemble_out(res.results)


# revision 25
# speedup vs baseline: 4.7196x; 2.4167x over previous
"""Causal self-attention on 8 TRN2 NeuronCores.

Sharding: 4-way data parallel over batch x 2-way tensor parallel over heads.
Core c handles batch b=c//2, head group g=c%2 (heads 8g..8g+8).

Per-core device kernel (bf16 matmuls, fp32 PSUM):
  1. QKV projection from host-pretransposed xT [C, T]:
     qT/kT head-dim-on-partitions ([128, T] tiles, head pairs stacked
     64+64 on partitions); V natural [T, 64/head] + ones column (V').
  2. Attention per head-PAIR, q processed in 512-chunks, k-block-major:
     S^T[k,q] for both heads via two row-tiled matmuls (K=64 each, PE
     runs them concurrently); one wide ACT exp (scale=1/8) over both
     heads; diagonal-block causal mask multiplied on DVE;
     Y'[65, q-chunk] += V'_j.T @ expS^T accumulates unnormalized y^T and
     the softmax denominator l (ones column).
     Normalize: DVE recip of l -> DMA partition-broadcast -> DVE mul
     writes y^T straight into the persistent proj-lhsT tiles yf.
  3. proj partial[q, :] = yT.T @ w_proj(group rows) + bp_eff (bp_eff
     host-folds b_proj/2 and the V-bias contribution bv @ w_proj).
  4. Pairwise ReduceScatter(add, f32) sums the two head groups and
     writes each core's query half DIRECTLY into the output tensor.

QKV/V/proj matmul groups are emitted as small "filler" units drained
between attention steps so the PE stays busy under the ACT-bound
exp stream.
"""
import collections
import numpy as np
import ml_dtypes

B, T, C = 4, 2048, 1024
H = 16
D = C // H  # 64
HPC = 8            # heads per core
GD = HPC * D       # 512 dims per core's head group

_CACHE = {}


def _build_nc(skip_rs=False, with_bias=False):
    import concourse.bass as bass
    import concourse.mybir as mybir
    import concourse.tile as tile
    from concourse import bacc
    from contextlib import ExitStack

    f32 = mybir.dt.float32
    bf16 = mybir.dt.bfloat16

    nc = bacc.Bacc("TRN2", target_bir_lowering=False, debug=False, num_devices=8)

    xT = nc.declare_dram_parameter("xT", [C, T], bf16, isOutput=False)
    wq = nc.declare_dram_parameter("wq", [C, GD], bf16, isOutput=False)
    wk = nc.declare_dram_parameter("wk", [C, GD], bf16, isOutput=False)
    wv = nc.declare_dram_parameter("wv", [C, GD], bf16, isOutput=False)
    wp = nc.declare_dram_parameter("wp", [GD, C], bf16, isOutput=False)
    bp = nc.declare_dram_parameter("bp", [C], f32, isOutput=False)
    if with_bias:
        bq = nc.declare_dram_parameter("bq", [GD], f32, isOutput=False)
        bk = nc.declare_dram_parameter("bk", [GD], f32, isOutput=False)
    out = nc.declare_dram_parameter("out", [T // 2, C], f32, isOutput=True)

    rs_in = nc.dram_tensor("rs_in", [T, C], f32)
    rs_out = nc.dram_tensor("rs_out", [T // 2, C], f32)

    NKB = T // 128   # 16 k-blocks
    NCC = C // 128   # 8 contraction chunks

    with tile.TileContext(nc) as tc, ExitStack() as S0:
        consts = S0.enter_context(tc.tile_pool(name="consts", bufs=1))
        wqkv = S0.enter_context(tc.tile_pool(name="wqkv", bufs=1))
        xp = S0.enter_context(tc.tile_pool(name="xp", bufs=1))
        wpp = S0.enter_context(tc.tile_pool(name="wpp", bufs=1))
        qk_pool = S0.enter_context(tc.tile_pool(name="qk", bufs=1))
        v_pool = S0.enter_context(tc.tile_pool(name="v", bufs=1))
        yf_pool = S0.enter_context(tc.tile_pool(name="yf", bufs=1))
        esp = S0.enter_context(tc.tile_pool(name="esp", bufs=3))
        rcp = S0.enter_context(tc.tile_pool(name="rcp", bufs=2))
        obp = S0.enter_context(tc.tile_pool(name="ob", bufs=2))
        # PSUM: sps 2x[128,1024]f32 (4 banks) + yps 2x[65,512] (2) + psb 2x[128,512] (2)
        sps = S0.enter_context(tc.tile_pool(name="sps", bufs=2, space="PSUM"))
        yps = S0.enter_context(tc.tile_pool(name="yps", bufs=1, space="PSUM"))
        psb = S0.enter_context(tc.tile_pool(name="psb", bufs=2, space="PSUM"))

        # ---- constants ----
        mask01 = consts.tile([128, 128], bf16, tag="mask")
        ones_t = consts.tile([128, D], bf16, tag="ones")
        nc.vector.memset(ones_t, 1.0)
        nc.gpsimd.memset(mask01, 1.0)
        # S^T[k, q] valid when k <= q: zero the strict lower triangle (k > q),
        # applied multiplicatively AFTER exp.
        nc.gpsimd.affine_select(
            out=mask01, in_=mask01,
            compare_op=mybir.AluOpType.is_ge, fill=0.0,
            base=0, pattern=[[1, 128]], channel_multiplier=-1,
        )
        bp_bc = consts.tile([128, C], f32, tag="bpb")
        nc.sync.dma_start(out=bp_bc, in_=bp.ap().partition_broadcast(128))
        if with_bias:
            bq_t = consts.tile([128, 4], f32, tag="bqt")
            bk_t = consts.tile([128, 4], f32, tag="bkt")
            for p in range(4):
                nc.sync.dma_start(
                    out=bq_t[:, p : p + 1],
                    in_=bq.ap()[128 * p : 128 * p + 128].rearrange("(p o) -> p o", o=1),
                )
                nc.sync.dma_start(
                    out=bk_t[:, p : p + 1],
                    in_=bk.ap()[128 * p : 128 * p + 128].rearrange("(p o) -> p o", o=1),
                )
            bqb = consts.tile([128, 4, 512], f32, tag="bqb")
            bkb = consts.tile([128, 4, 512], f32, tag="bkb")
            nc.vector.memset(bqb, 0.0)
            nc.vector.memset(bkb, 0.0)
            for p in range(4):
                nc.vector.tensor_scalar_add(bqb[:, p, :], bqb[:, p, :], bq_t[:, p : p + 1])
                nc.vector.tensor_scalar_add(bkb[:, p, :], bkb[:, p, :], bk_t[:, p : p + 1])

        # ---- persistent tiles ----
        wq_t = [wqkv.tile([128, GD], bf16, tag=f"wq{i}", name=f"wqt{i}") for i in range(NCC)]
        wk_t = [wqkv.tile([128, GD], bf16, tag=f"wk{i}", name=f"wkt{i}") for i in range(NCC)]
        wv_t = [wqkv.tile([128, GD], bf16, tag=f"wv{i}", name=f"wvt{i}") for i in range(NCC)]
        xT_t = [xp.tile([128, T], bf16, tag=f"x{i}", name=f"x{i}") for i in range(NCC)]
        wp_t = [wpp.tile([128, C], bf16, tag=f"wp{i}", name=f"wp{i}") for i in range(4)]
        qT = [qk_pool.tile([128, T], bf16, tag=f"qT{p}", name=f"qT{p}") for p in range(4)]
        kT = [qk_pool.tile([128, T], bf16, tag=f"kT{p}", name=f"kT{p}") for p in range(4)]
        vp = [v_pool.tile([128, HPC * 65], bf16, tag=f"vp{tb}", name=f"vp{tb}") for tb in range(NKB)]
        yf = [yf_pool.tile([128, T], bf16, tag=f"yf{p}", name=f"yf{p}") for p in range(4)]

        # x columns [0:1024] unblock qkT(0) t4=0,1 and V(0..7); the rest
        # streams in behind them.
        for i in range(NCC):
            sl = slice(128 * i, 128 * i + 128)
            nc.sync.dma_start(out=wq_t[i], in_=wq.ap()[sl, :])
            nc.sync.dma_start(out=wk_t[i], in_=wk.ap()[sl, :])
            nc.sync.dma_start(out=xT_t[i][:, 0:1024], in_=xT.ap()[sl, 0:1024])
        for i in range(NCC):
            sl = slice(128 * i, 128 * i + 128)
            nc.sync.dma_start(out=wv_t[i], in_=wv.ap()[sl, :])
            nc.sync.dma_start(out=xT_t[i][:, 1024:2048], in_=xT.ap()[sl, 1024:2048])
        for i in range(4):
            nc.sync.dma_start(out=wp_t[i], in_=wp.ap()[128 * i : 128 * i + 128, :])
        # ones columns of V' (written once; V evac fills only [0:64] per head)
        for tb in range(NKB):
            nc.vector.memset(
                vp[tb].rearrange("p (h e) -> p h e", e=65)[:, :, D : D + 1], 1.0
            )

        # ---- emission thunk generators (filler units) ----
        def qkT_group(is_k, p, t4):
            """One [128,512] psum group of the q/k projection -> 9 thunks."""
            w_t = wk_t if is_k else wq_t
            dst = (kT if is_k else qT)[p]
            cell = {}

            def mk_mm(cc):
                def f():
                    if cc == 0:
                        cell["ps"] = psb.tile([128, 512], f32, tag="pf", name="pf")
                    nc.tensor.matmul(
                        cell["ps"],
                        w_t[cc][:, 128 * p : 128 * p + 128],
                        xT_t[cc][:, 512 * t4 : 512 * t4 + 512],
                        start=(cc == 0), stop=(cc == NCC - 1),
                    )
                return f

            def evac():
                d = dst[:, 512 * t4 : 512 * t4 + 512]
                if with_bias:
                    bb = (bkb if is_k else bqb)[:, p, :]
                    nc.vector.tensor_add(d, cell["ps"], bb)
                else:
                    nc.vector.tensor_copy(d, cell["ps"])

            return [mk_mm(cc) for cc in range(NCC)] + [evac]

        def v_group(tb):
            cell = {}

            def mk_mm(cc):
                def f():
                    if cc == 0:
                        cell["ps"] = psb.tile([128, GD], f32, tag="pf", name="pv")
                    nc.tensor.matmul(
                        cell["ps"],
                        xT_t[cc][:, 128 * tb : 128 * tb + 128],
                        wv_t[cc],
                        start=(cc == 0), stop=(cc == NCC - 1),
                    )
                return f

            def evac():
                v3 = vp[tb].rearrange("p (h e) -> p h e", e=65)
                nc.vector.tensor_copy(
                    v3[:, :, 0:D], cell["ps"].rearrange("p (h e) -> p h e", e=D)
                )

            return [mk_mm(cc) for cc in range(NCC)] + [evac]

        def proj_group(qq):
            """qq is the GLOBAL 128-row q block (0..15); 2 psum halves."""
            thunks = []
            cell = {}

            def alloc_ob():
                cell["ob"] = obp.tile([128, C], f32, tag="ob", name="ob")

            for cc2 in range(2):
                def mk_mm(dd, cc2=cc2):
                    def f():
                        if dd == 0:
                            if cc2 == 0:
                                alloc_ob()
                            cell["ps"] = psb.tile([128, 512], f32, tag="pf", name="pp")
                        nc.tensor.matmul(
                            cell["ps"],
                            yf[dd][:, 128 * qq : 128 * qq + 128],
                            wp_t[dd][:, 512 * cc2 : 512 * cc2 + 512],
                            start=(dd == 0), stop=(dd == 3),
                        )
                    return f

                def evac(cc2=cc2):
                    nc.vector.tensor_add(
                        cell["ob"][:, 512 * cc2 : 512 * cc2 + 512],
                        cell["ps"],
                        bp_bc[:, 512 * cc2 : 512 * cc2 + 512],
                    )

                thunks += [mk_mm(dd) for dd in range(4)] + [evac]

            def dma():
                nc.sync.dma_start(
                    out=rs_in.ap()[128 * qq : 128 * qq + 128, :], in_=cell["ob"]
                )

            thunks.append(dma)
            return thunks

        # drain pulls from the first non-empty deque in drain_sources
        fillers = collections.deque()
        drain_sources = [fillers]

        def drain(n):
            for _ in range(n):
                for q in drain_sources:
                    if q:
                        q.popleft()()
                        break
                else:
                    return

        def drain_all():
            for q in drain_sources:
                while q:
                    q.popleft()()

        # ---- attention ----
        def attn_pair(m, p, after_cl=None):
            h0, h1 = 2 * p, 2 * p + 1
            for cl in (2 * m, 2 * m + 1):
                Y0 = yps.tile([65, 512], f32, tag="yh", name="yh")
                Y1 = yps.tile([65, 512], f32, tag="yh2", name="yh2")
                nj = 4 * cl + 4
                prev = None  # (es, j, qa_l, w)
                for j in range(nj):
                    qa_l = max(0, 128 * j - 512 * cl)
                    w = 512 - qa_l
                    qsl = slice(512 * cl + qa_l, 512 * cl + 512)
                    ksl = slice(128 * j, 128 * j + 128)
                    st = sps.tile([128, 1024], f32, tag="s", name="st")
                    nc.tensor.matmul(
                        st[:, 0:w], kT[p][0:64, ksl], qT[p][0:64, qsl],
                        start=True, stop=True,
                    )
                    nc.tensor.matmul(
                        st[:, 512 : 512 + w], kT[p][64:128, ksl], qT[p][64:128, qsl],
                        start=True, stop=True,
                    )
                    es = esp.tile([128, 1024], bf16, tag="es", name="es")
                    nc.scalar.activation(
                        es.rearrange("pp (h q) -> pp h q", h=2)[:, :, 0:w],
                        st.rearrange("pp (h q) -> pp h q", h=2)[:, :, 0:w],
                        mybir.ActivationFunctionType.Exp,
                        bias=0.0, scale=0.125,
                    )
                    if j >= 4 * cl:  # diagonal block: first 128 cols of region
                        nc.vector.tensor_mul(es[:, 0:128], es[:, 0:128], mask01)
                        nc.vector.tensor_mul(es[:, 512:640], es[:, 512:640], mask01)
                    drain(1)
                    if prev is not None:
                        emit_av(prev, nj, Y0, Y1, h0, h1)
                        drain(1)
                    prev = (es, j, qa_l, w)
                emit_av(prev, nj, Y0, Y1, h0, h1)
                # normalize both heads for this q-chunk: recip of l, rank-1
                # PE broadcast across the 64 d-partitions, then scale.
                for Y, r in ((Y0, 0), (Y1, 1)):
                    rb = rcp.tile([1, 512], bf16, tag="rb", name="rb")
                    with nc.allow_low_precision(reason="softmax denom bf16"):
                        nc.vector.reciprocal(rb, Y[64:65, :])
                    rbc = sps.tile([64, 512], f32, tag="s", name="rbc")
                    nc.tensor.matmul(
                        rbc, ones_t[0:1, 0:64], rb[0:1, :], start=True, stop=True
                    )
                    rbs = rcp.tile([64, 512], f32, tag="rbs", name="rbs")
                    nc.vector.tensor_copy(rbs, rbc)
                    nc.vector.tensor_mul(
                        yf[p][64 * r : 64 * r + 64, 512 * cl : 512 * cl + 512],
                        Y[0:64, :],
                        rbs,
                    )
                drain(2)
                if after_cl is not None:
                    after_cl(cl)

        def emit_av(prev, nj, Y0, Y1, h0, h1):
            es, j, qa_l, w = prev
            last = j == nj - 1
            nc.tensor.matmul(
                Y0[:, qa_l:512], vp[j][:, 65 * h0 : 65 * h0 + 65], es[:, 0:w],
                start=(j == 0), stop=last, skip_group_check=True,
            )
            nc.tensor.matmul(
                Y1[:, qa_l:512], vp[j][:, 65 * h1 : 65 * h1 + 65], es[:, 512 : 512 + w],
                start=(j == 0), stop=last, skip_group_check=True,
            )

        def emit_rs(m):
            if skip_rs:
                nc.sync.dma_start(
                    out=out.ap()[512 * m : 512 * m + 512, :],
                    in_=rs_in.ap()[1024 * m : 1024 * m + 512, :],
                )
                return
            nc.gpsimd.collective_compute(
                "ReduceScatter",
                mybir.AluOpType.add,
                ins=[rs_in.ap()[1024 * m : 1024 * m + 1024, :]],
                outs=[rs_out.ap()[512 * m : 512 * m + 512, :]],
                replica_groups=[[0, 1], [2, 3], [4, 5], [6, 7]],
            )
            nc.sync.dma_start(
                out=out.ap()[512 * m : 512 * m + 512, :],
                in_=rs_out.ap()[512 * m : 512 * m + 512, :],
            )

        # ---- emission schedule ----
        # Lead-in: q/k cols [0:1024] for pair 0 and V for the first 8
        # k-blocks — exactly what attn(0,0) consumes.
        for t4 in (0, 1):
            for t in qkT_group(False, 0, t4) + qkT_group(True, 0, t4):
                t()
        for tb in range(8):
            for t in v_group(tb):
                t()

        # Fillers for the m=0 attention phase. attn(0,p) needs only pair
        # p's t4=0,1 (fq[p], drained with priority); the t4=2,3 halves and
        # V(8..15) are only needed for m=1 and fill PE gaps.
        fq = {p: collections.deque() for p in (1, 2, 3)}
        for p in (1, 2, 3):
            for t4 in (0, 1):
                fq[p].extend(qkT_group(False, p, t4))
                fq[p].extend(qkT_group(True, p, t4))
        for tb in range(8, NKB):
            fillers.extend(v_group(tb))
        for p in range(4):
            for t4 in (2, 3):
                fillers.extend(qkT_group(False, p, t4))
                fillers.extend(qkT_group(True, p, t4))

        for p in range(4):
            if p > 0:
                # force-emit anything pair p still needs
                while fq[p]:
                    fq[p].popleft()()
            drain_sources[:] = (
                [fq[p + 1], fillers] if p + 1 in fq else [fillers]
            )
            attn_pair(0, p)

        drain_sources[:] = [fillers]
        drain_all()
        for qq in range(7):
            fillers.extend(proj_group(qq))

        attn_pair(1, 0)
        fillers.extend(proj_group(7))
        attn_pair(1, 1)
        drain_all()
        emit_rs(0)
        attn_pair(1, 2)

        def after_cl(cl):
            if cl == 2:
                for qq in range(8, 12):
                    fillers.extend(proj_group(qq))

        attn_pair(1, 3, after_cl=after_cl)
        drain_all()
        for qq in range(12, 16):
            for t in proj_group(qq):
                t()
        emit_rs(1)

    nc.finalize()
    return nc


def get_nc(skip_rs=False, with_bias=False):
    key = ("nc", skip_rs, with_bias)
    if key not in _CACHE:
        _CACHE[key] = _build_nc(skip_rs, with_bias)
    return _CACHE[key]


def build_in_maps(x, w_attn, b_attn, w_proj, b_proj, with_bias=False):
    bf = ml_dtypes.bfloat16
    x = np.asarray(x, dtype=np.float32)
    w_attn = np.asarray(w_attn, dtype=np.float32)
    b_attn = np.asarray(b_attn, dtype=np.float32)
    w_proj = np.asarray(w_proj, dtype=np.float32)
    b_proj = np.asarray(b_proj, dtype=np.float32)

    in_maps = []
    for c in range(8):
        b, g = c // 2, c % 2
        sl = slice(GD * g, GD * g + GD)
        wp_g = w_proj[GD * g : GD * g + GD, :]
        bv_g = b_attn[2 * C :][sl]
        # fold b_proj/2 and the V-bias contribution into one proj bias
        bp_eff = (0.5 * b_proj + bv_g @ wp_g).astype(np.float32)
        m = {
            "xT": np.ascontiguousarray(x[b].T).astype(bf),
            "wq": np.ascontiguousarray(w_attn[:, 0 * C :][:, sl]).astype(bf),
            "wk": np.ascontiguousarray(w_attn[:, 1 * C :][:, sl]).astype(bf),
            "wv": np.ascontiguousarray(w_attn[:, 2 * C :][:, sl]).astype(bf),
            "wp": np.ascontiguousarray(wp_g).astype(bf),
            "bp": bp_eff,
        }
        if with_bias:
            m["bq"] = np.ascontiguousarray(b_attn[0 * C :][sl])
            m["bk"] = np.ascontiguousarray(b_attn[1 * C :][sl])
        in_maps.append(m)
    return in_maps


def assemble_out(results):
    # core with parity g owns q in [512g, 512g+512) of each 1024-half
    out = np.empty((B, T, C), dtype=np.float32)
    for c in range(8):
        b, g = c // 2, c % 2
        piece = results[c]["out"]  # [1024, C]
        out[b, 512 * g : 512 * g + 512, :] = piece[0:512]
        out[b, 1024 + 512 * g : 1024 + 512 * g + 512, :] = piece[512:1024]
    return out


def kernel(x, w_attn, b_attn, w_proj, b_proj):
    from concourse.bass_utils import run_bass_kernel_spmd

    b_attn_np = np.asarray(b_attn, dtype=np.float32)
    with_bias = bool(np.any(b_attn_np[: 2 * C] != 0.0))
    nc = get_nc(with_bias=with_bias)
    in_maps = build_in_maps(x, w_attn, b_attn, w_proj, b_proj, with_bias=with_bias)
    res = run_bass_kernel_spmd(nc, in_maps, core_ids=list(range(8)))
    return assemble_out(res.results)


# revision 33
# speedup vs baseline: 5.1624x; 1.0938x over previous
"""Causal self-attention on 8 TRN2 NeuronCores.

Sharding: 4-way data parallel over batch x 2-way tensor parallel over heads.
Core c handles batch b=c//2, head group g=c%2 (heads 8g..8g+8).

Per-core device kernel (bf16 matmuls, fp32 PSUM):
  1. QKV projection from host-pretransposed xT [C, T]:
     qT/kT head-dim-on-partitions ([128, T] tiles, head pairs stacked
     64+64 on partitions); V natural [T, 64/head] + ones column (V').
  2. Attention per head-PAIR, q processed in 512-chunks, k-block-major:
     S^T[k,q] for both heads via two row-tiled matmuls (K=64 each, PE
     runs them concurrently); one wide ACT exp (scale=1/8) over both
     heads; diagonal-block causal mask multiplied on DVE;
     Y'[65, q-chunk] += V'_j.T @ expS^T accumulates unnormalized y^T and
     the softmax denominator l (ones column).
     Normalize: DVE recip of l -> DMA partition-broadcast -> DVE mul
     writes y^T straight into the persistent proj-lhsT tiles yf.
  3. proj partial[q, :] = yT.T @ w_proj(group rows) + bp_eff (bp_eff
     host-folds b_proj/2 and the V-bias contribution bv @ w_proj).
  4. Pairwise ReduceScatter(add, f32) sums the two head groups and
     writes each core's query half DIRECTLY into the output tensor.

QKV/V/proj matmul groups are emitted as small "filler" units drained
between attention steps so the PE stays busy under the ACT-bound
exp stream.
"""
import collections
import numpy as np
import ml_dtypes

B, T, C = 4, 2048, 1024
H = 16
D = C // H  # 64
HPC = 8            # heads per core
GD = HPC * D       # 512 dims per core's head group

_CACHE = {}


def _build_nc(skip_rs=False, with_bias=False):
    import concourse.bass as bass
    import concourse.mybir as mybir
    import concourse.tile as tile
    from concourse import bacc
    from contextlib import ExitStack

    f32 = mybir.dt.float32
    bf16 = mybir.dt.bfloat16

    nc = bacc.Bacc("TRN2", target_bir_lowering=False, debug=False, num_devices=8)

    xT = nc.declare_dram_parameter("xT", [C, T], bf16, isOutput=False)
    wq = nc.declare_dram_parameter("wq", [C, GD], bf16, isOutput=False)
    wk = nc.declare_dram_parameter("wk", [C, GD], bf16, isOutput=False)
    wv = nc.declare_dram_parameter("wv", [C, GD], bf16, isOutput=False)
    wp = nc.declare_dram_parameter("wp", [GD, C], bf16, isOutput=False)
    bp = nc.declare_dram_parameter("bp", [C], f32, isOutput=False)
    if with_bias:
        bq = nc.declare_dram_parameter("bq", [GD], f32, isOutput=False)
        bk = nc.declare_dram_parameter("bk", [GD], f32, isOutput=False)
    out = nc.declare_dram_parameter("out", [T // 2, C], f32, isOutput=True)

    rs_in = nc.dram_tensor("rs_in", [T, C], f32)
    rs_out = nc.dram_tensor("rs_out", [T // 2, C], f32)

    NKB = T // 128   # 16 k-blocks
    NCC = C // 128   # 8 contraction chunks

    with tile.TileContext(nc) as tc, ExitStack() as S0:
        consts = S0.enter_context(tc.tile_pool(name="consts", bufs=1))
        wqkv = S0.enter_context(tc.tile_pool(name="wqkv", bufs=1))
        xp = S0.enter_context(tc.tile_pool(name="xp", bufs=1))
        wpp = S0.enter_context(tc.tile_pool(name="wpp", bufs=1))
        qk_pool = S0.enter_context(tc.tile_pool(name="qk", bufs=1))
        v_pool = S0.enter_context(tc.tile_pool(name="v", bufs=1))
        yf_pool = S0.enter_context(tc.tile_pool(name="yf", bufs=1))
        esp = S0.enter_context(tc.tile_pool(name="esp", bufs=3))
        rcp = S0.enter_context(tc.tile_pool(name="rcp", bufs=2))
        obp = S0.enter_context(tc.tile_pool(name="ob", bufs=2))
        # PSUM: sps 2x[128,1024]f32 (4 banks) + yps 2x[65,512] (2) +
        # psb 2x[128,512] (2) = 8 banks
        sps = S0.enter_context(tc.tile_pool(name="sps", bufs=2, space="PSUM"))
        yps = S0.enter_context(tc.tile_pool(name="yps", bufs=1, space="PSUM"))
        psb = S0.enter_context(tc.tile_pool(name="psb", bufs=2, space="PSUM"))

        # ---- constants ----
        mask01 = consts.tile([128, 128], bf16, tag="mask")
        nc.gpsimd.memset(mask01, 1.0)
        # S^T[k, q] valid when k <= q: zero the strict lower triangle (k > q),
        # applied multiplicatively AFTER exp.
        nc.gpsimd.affine_select(
            out=mask01, in_=mask01,
            compare_op=mybir.AluOpType.is_ge, fill=0.0,
            base=0, pattern=[[1, 128]], channel_multiplier=-1,
        )
        bp_bc = consts.tile([128, C], f32, tag="bpb")
        nc.sync.dma_start(out=bp_bc, in_=bp.ap().partition_broadcast(128))
        if with_bias:
            bq_t = consts.tile([128, 4], f32, tag="bqt")
            bk_t = consts.tile([128, 4], f32, tag="bkt")
            for p in range(4):
                nc.sync.dma_start(
                    out=bq_t[:, p : p + 1],
                    in_=bq.ap()[128 * p : 128 * p + 128].rearrange("(p o) -> p o", o=1),
                )
                nc.sync.dma_start(
                    out=bk_t[:, p : p + 1],
                    in_=bk.ap()[128 * p : 128 * p + 128].rearrange("(p o) -> p o", o=1),
                )
            bqb = consts.tile([128, 4, 512], f32, tag="bqb")
            bkb = consts.tile([128, 4, 512], f32, tag="bkb")
            nc.vector.memset(bqb, 0.0)
            nc.vector.memset(bkb, 0.0)
            for p in range(4):
                nc.vector.tensor_scalar_add(bqb[:, p, :], bqb[:, p, :], bq_t[:, p : p + 1])
                nc.vector.tensor_scalar_add(bkb[:, p, :], bkb[:, p, :], bk_t[:, p : p + 1])

        # ---- persistent tiles ----
        wq_t = [wqkv.tile([128, GD], bf16, tag=f"wq{i}", name=f"wqt{i}") for i in range(NCC)]
        wk_t = [wqkv.tile([128, GD], bf16, tag=f"wk{i}", name=f"wkt{i}") for i in range(NCC)]
        wv_t = [wqkv.tile([128, GD], bf16, tag=f"wv{i}", name=f"wvt{i}") for i in range(NCC)]
        xT_t = [xp.tile([128, T], bf16, tag=f"x{i}", name=f"x{i}") for i in range(NCC)]
        wp_t = [wpp.tile([128, C], bf16, tag=f"wp{i}", name=f"wp{i}") for i in range(4)]
        qT = [qk_pool.tile([128, T], bf16, tag=f"qT{p}", name=f"qT{p}") for p in range(4)]
        kT = [qk_pool.tile([128, T], bf16, tag=f"kT{p}", name=f"kT{p}") for p in range(4)]
        vp = [v_pool.tile([128, HPC * 65], bf16, tag=f"vp{tb}", name=f"vp{tb}") for tb in range(NKB)]
        yf = [yf_pool.tile([128, T], bf16, tag=f"yf{p}", name=f"yf{p}") for p in range(4)]

        # x columns [0:512] unblock qkT(0) t4=0 as early as possible;
        # later column quarters stream in behind.
        for i in range(NCC):
            sl = slice(128 * i, 128 * i + 128)
            nc.sync.dma_start(out=wq_t[i], in_=wq.ap()[sl, :])
            nc.sync.dma_start(out=wk_t[i], in_=wk.ap()[sl, :])
            nc.sync.dma_start(out=xT_t[i][:, 0:512], in_=xT.ap()[sl, 0:512])
        for i in range(NCC):
            sl = slice(128 * i, 128 * i + 128)
            nc.sync.dma_start(out=xT_t[i][:, 512:1024], in_=xT.ap()[sl, 512:1024])
        for i in range(NCC):
            sl = slice(128 * i, 128 * i + 128)
            nc.sync.dma_start(out=wv_t[i], in_=wv.ap()[sl, :])
            nc.sync.dma_start(out=xT_t[i][:, 1024:2048], in_=xT.ap()[sl, 1024:2048])
        for i in range(4):
            nc.sync.dma_start(out=wp_t[i], in_=wp.ap()[128 * i : 128 * i + 128, :])
        # ones columns of V' (written once; V evac fills only [0:64] per head)
        for tb in range(NKB):
            nc.vector.memset(
                vp[tb].rearrange("p (h e) -> p h e", e=65)[:, :, D : D + 1], 1.0
            )

        # ---- emission thunk generators (filler units) ----
        def qkT_group(is_k, p, t4):
            """One [128,512] psum group of the q/k projection -> 9 thunks."""
            w_t = wk_t if is_k else wq_t
            dst = (kT if is_k else qT)[p]
            cell = {}

            def mk_mm(cc):
                def f():
                    if cc == 0:
                        cell["ps"] = psb.tile([128, 512], f32, tag="pf", name="pf")
                    nc.tensor.matmul(
                        cell["ps"],
                        w_t[cc][:, 128 * p : 128 * p + 128],
                        xT_t[cc][:, 512 * t4 : 512 * t4 + 512],
                        start=(cc == 0), stop=(cc == NCC - 1),
                    )
                return f

            def evac():
                d = dst[:, 512 * t4 : 512 * t4 + 512]
                if with_bias:
                    bb = (bkb if is_k else bqb)[:, p, :]
                    nc.vector.tensor_add(d, cell["ps"], bb)
                else:
                    nc.vector.tensor_copy(d, cell["ps"])

            return [mk_mm(cc) for cc in range(NCC)] + [evac]

        def v_group(tb):
            cell = {}

            def mk_mm(cc):
                def f():
                    if cc == 0:
                        cell["ps"] = psb.tile([128, GD], f32, tag="pf", name="pv")
                    nc.tensor.matmul(
                        cell["ps"],
                        xT_t[cc][:, 128 * tb : 128 * tb + 128],
                        wv_t[cc],
                        start=(cc == 0), stop=(cc == NCC - 1),
                    )
                return f

            def evac():
                v3 = vp[tb].rearrange("p (h e) -> p h e", e=65)
                nc.vector.tensor_copy(
                    v3[:, :, 0:D], cell["ps"].rearrange("p (h e) -> p h e", e=D)
                )

            return [mk_mm(cc) for cc in range(NCC)] + [evac]

        def proj_group(qq):
            """qq is the GLOBAL 128-row q block (0..15); 2 psum halves."""
            thunks = []
            cell = {}

            def alloc_ob():
                cell["ob"] = obp.tile([128, C], f32, tag="ob", name="ob")

            for cc2 in range(2):
                def mk_mm(dd, cc2=cc2):
                    def f():
                        if dd == 0:
                            if cc2 == 0:
                                alloc_ob()
                            cell["ps"] = psb.tile([128, 512], f32, tag="pf", name="pp")
                        nc.tensor.matmul(
                            cell["ps"],
                            yf[dd][:, 128 * qq : 128 * qq + 128],
                            wp_t[dd][:, 512 * cc2 : 512 * cc2 + 512],
                            start=(dd == 0), stop=(dd == 3),
                        )
                    return f

                def evac(cc2=cc2):
                    nc.vector.tensor_add(
                        cell["ob"][:, 512 * cc2 : 512 * cc2 + 512],
                        cell["ps"],
                        bp_bc[:, 512 * cc2 : 512 * cc2 + 512],
                    )

                thunks += [mk_mm(dd) for dd in range(4)] + [evac]

            def dma():
                nc.sync.dma_start(
                    out=rs_in.ap()[128 * qq : 128 * qq + 128, :], in_=cell["ob"]
                )

            thunks.append(dma)
            return thunks

        # drain pulls from the first non-empty deque in drain_sources
        fillers = collections.deque()
        drain_sources = [fillers]

        def drain(n):
            for _ in range(n):
                for q in drain_sources:
                    if q:
                        q.popleft()()
                        break
                else:
                    return

        def drain_all():
            for q in drain_sources:
                while q:
                    q.popleft()()

        # ---- attention ----
        # Normalize work for a finished q-chunk is DEFERRED until the next
        # chunk's first S/exp is in flight, so the recip->broadcast chain
        # never sits between ACT and its next exp input.
        pending_norm = []

        def flush_norm():
            while pending_norm:
                pending_norm.pop(0)()

        def attn_pair(m, p, after_cl=None):
            h0, h1 = 2 * p, 2 * p + 1
            for cl in (2 * m, 2 * m + 1):
                Y0 = yps.tile([65, 512], f32, tag="yh", name="yh")
                Y1 = yps.tile([65, 512], f32, tag="yh2", name="yh2")
                nj = 4 * cl + 4
                prev = None  # (es, j, qa_l, w)
                for j in range(nj):
                    qa_l = max(0, 128 * j - 512 * cl)
                    w = 512 - qa_l
                    qsl = slice(512 * cl + qa_l, 512 * cl + 512)
                    ksl = slice(128 * j, 128 * j + 128)
                    st = sps.tile([128, 1024], f32, tag="s", name="st")
                    nc.tensor.matmul(
                        st[:, 0:w], kT[p][0:64, ksl], qT[p][0:64, qsl],
                        start=True, stop=True,
                    )
                    nc.tensor.matmul(
                        st[:, 512 : 512 + w], kT[p][64:128, ksl], qT[p][64:128, qsl],
                        start=True, stop=True,
                    )
                    es = esp.tile([128, 1024], bf16, tag="es", name="es")
                    nc.scalar.activation(
                        es.rearrange("pp (h q) -> pp h q", h=2)[:, :, 0:w],
                        st.rearrange("pp (h q) -> pp h q", h=2)[:, :, 0:w],
                        mybir.ActivationFunctionType.Exp,
                        bias=0.0, scale=0.125,
                    )
                    if j >= 4 * cl:  # diagonal block: first 128 cols of region
                        nc.vector.tensor_mul(es[:, 0:128], es[:, 0:128], mask01)
                        nc.vector.tensor_mul(es[:, 512:640], es[:, 512:640], mask01)
                    if j == 0:
                        flush_norm()
                    drain(1)
                    if prev is not None:
                        emit_av(prev, nj, Y0, Y1, h0, h1)
                        drain(1)
                    prev = (es, j, qa_l, w)
                emit_av(prev, nj, Y0, Y1, h0, h1)

                def norm(Y0=Y0, Y1=Y1, p=p, cl=cl):
                    # recip of l, GPSIMD broadcast across the 64
                    # d-partitions, then scale into the proj lhsT tiles.
                    for Y, r in ((Y0, 0), (Y1, 1)):
                        rb = rcp.tile([1, 512], bf16, tag="rb", name="rb")
                        with nc.allow_low_precision(reason="softmax denom bf16"):
                            nc.vector.reciprocal(rb, Y[64:65, :])
                        rbs = rcp.tile([64, 512], bf16, tag="rbs", name="rbs")
                        nc.gpsimd.partition_broadcast(rbs, rb[0:1, :], channels=64)
                        nc.vector.tensor_mul(
                            yf[p][64 * r : 64 * r + 64, 512 * cl : 512 * cl + 512],
                            Y[0:64, :],
                            rbs,
                        )

                pending_norm.append(norm)
                drain(2)
                if after_cl is not None:
                    after_cl(cl)

        def emit_av(prev, nj, Y0, Y1, h0, h1):
            es, j, qa_l, w = prev
            last = j == nj - 1
            nc.tensor.matmul(
                Y0[:, qa_l:512], vp[j][:, 65 * h0 : 65 * h0 + 65], es[:, 0:w],
                start=(j == 0), stop=last, skip_group_check=True,
            )
            nc.tensor.matmul(
                Y1[:, qa_l:512], vp[j][:, 65 * h1 : 65 * h1 + 65], es[:, 512 : 512 + w],
                start=(j == 0), stop=last, skip_group_check=True,
            )

        def emit_rs(m):
            if skip_rs:
                nc.sync.dma_start(
                    out=out.ap()[512 * m : 512 * m + 512, :],
                    in_=rs_in.ap()[1024 * m : 1024 * m + 512, :],
                )
                return
            nc.gpsimd.collective_compute(
                "ReduceScatter",
                mybir.AluOpType.add,
                ins=[rs_in.ap()[1024 * m : 1024 * m + 1024, :]],
                outs=[rs_out.ap()[512 * m : 512 * m + 512, :]],
                replica_groups=[[0, 1], [2, 3], [4, 5], [6, 7]],
            )
            nc.sync.dma_start(
                out=out.ap()[512 * m : 512 * m + 512, :],
                in_=rs_out.ap()[512 * m : 512 * m + 512, :],
            )

        # ---- emission schedule ----
        # Lead-in: q/k cols [0:1024] for pair 0 and V for the first 8
        # k-blocks — exactly what attn(0,0) consumes.
        for t4 in (0, 1):
            for t in qkT_group(False, 0, t4) + qkT_group(True, 0, t4):
                t()
        for tb in range(8):
            for t in v_group(tb):
                t()

        # Fillers for the m=0 attention phase. attn(0,p) needs only pair
        # p's t4=0,1 (fq[p], drained with priority); the t4=2,3 halves and
        # V(8..15) are only needed for m=1 and fill PE gaps.
        fq = {p: collections.deque() for p in (1, 2, 3)}
        for p in (1, 2, 3):
            for t4 in (0, 1):
                fq[p].extend(qkT_group(False, p, t4))
                fq[p].extend(qkT_group(True, p, t4))
        for tb in range(8, NKB):
            fillers.extend(v_group(tb))
        for p in range(4):
            for t4 in (2, 3):
                fillers.extend(qkT_group(False, p, t4))
                fillers.extend(qkT_group(True, p, t4))

        for p in range(4):
            if p > 0:
                # force-emit anything pair p still needs
                while fq[p]:
                    fq[p].popleft()()
            drain_sources[:] = (
                [fq[p + 1], fillers] if p + 1 in fq else [fillers]
            )
            attn_pair(0, p)

        drain_sources[:] = [fillers]
        drain_all()
        for qq in range(7):
            fillers.extend(proj_group(qq))

        attn_pair(1, 0)
        fillers.extend(proj_group(7))
        attn_pair(1, 1)
        drain_all()
        emit_rs(0)
        attn_pair(1, 2)

        def after_cl(cl):
            if cl == 2:
                for qq in range(8, 12):
                    fillers.extend(proj_group(qq))

        attn_pair(1, 3, after_cl=after_cl)
        flush_norm()
        drain_all()
        for qq in range(12, 16):
            for t in proj_group(qq):
                t()
        emit_rs(1)

    nc.finalize()
    return nc


def get_nc(skip_rs=False, with_bias=False):
    key = ("nc", skip_rs, with_bias)
    if key not in _CACHE:
        _CACHE[key] = _build_nc(skip_rs, with_bias)
    return _CACHE[key]


def build_in_maps(x, w_attn, b_attn, w_proj, b_proj, with_bias=False):
    bf = ml_dtypes.bfloat16
    x = np.asarray(x, dtype=np.float32)
    w_attn = np.asarray(w_attn, dtype=np.float32)
    b_attn = np.asarray(b_attn, dtype=np.float32)
    w_proj = np.asarray(w_proj, dtype=np.float32)
    b_proj = np.asarray(b_proj, dtype=np.float32)

    in_maps = []
    for c in range(8):
        b, g = c // 2, c % 2
        sl = slice(GD * g, GD * g + GD)
        wp_g = w_proj[GD * g : GD * g + GD, :]
        bv_g = b_attn[2 * C :][sl]
        # fold b_proj/2 and the V-bias contribution into one proj bias
        bp_eff = (0.5 * b_proj + bv_g @ wp_g).astype(np.float32)
        m = {
            "xT": np.ascontiguousarray(x[b].T).astype(bf),
            "wq": np.ascontiguousarray(w_attn[:, 0 * C :][:, sl]).astype(bf),
            "wk": np.ascontiguousarray(w_attn[:, 1 * C :][:, sl]).astype(bf),
            "wv": np.ascontiguousarray(w_attn[:, 2 * C :][:, sl]).astype(bf),
            "wp": np.ascontiguousarray(wp_g).astype(bf),
            "bp": bp_eff,
        }
        if with_bias:
            m["bq"] = np.ascontiguousarray(b_attn[0 * C :][sl])
            m["bk"] = np.ascontiguousarray(b_attn[1 * C :][sl])
        in_maps.append(m)
    return in_maps


def assemble_out(results):
    # core with parity g owns q in [512g, 512g+512) of each 1024-half
    out = np.empty((B, T, C), dtype=np.float32)
    for c in range(8):
        b, g = c // 2, c % 2
        piece = results[c]["out"]  # [1024, C]
        out[b, 512 * g : 512 * g + 512, :] = piece[0:512]
        out[b, 1024 + 512 * g : 1024 + 512 * g + 512, :] = piece[512:1024]
    return out


def kernel(x, w_attn, b_attn, w_proj, b_proj):
    from concourse.bass_utils import run_bass_kernel_spmd

    b_attn_np = np.asarray(b_attn, dtype=np.float32)
    with_bias = bool(np.any(b_attn_np[: 2 * C] != 0.0))
    nc = get_nc(with_bias=with_bias)
    in_maps = build_in_maps(x, w_attn, b_attn, w_proj, b_proj, with_bias=with_bias)
    res = run_bass_kernel_spmd(nc, in_maps, core_ids=list(range(8)))
    return assemble_out(res.results)
